# revision 19
# baseline (speedup 1.0000x reference)
"""AdditiveAttention on Trainium2 (Bass/Tile), 8 cores, valid_lens-aware resharding.

Reference per batch b:
  q = queries @ Wq; k = keys @ Wk
  scores[i,j] = wv . tanh(q[i] + k[j]); masked softmax over j; out = attn @ values

Masked columns (j >= valid_len) contribute exactly zero, so only
ceil(valid_len/128) j-blocks per batch need computing. Work units of
(batch, 64 query rows) x (128-col j-block) are bin-packed across the 8 cores:
each core gets two 64-row units (A rows->u=0, B->u=1) and exactly S j-block
slots total (pairs chosen so sums are equal; short cores get zero-key,
-1e6-masked pad slots). One SPMD program: all per-core variation lives in
input DATA (slot-gathered keys/values/mask, slot-replicated qT, one-hot wv
stationaries); instructions are identical on every core.

Row permutation i_phys(u, l) = 32*(l%4) + 16*u + l//4 puts consecutive l on
disjoint PE column groups (4-way tile_position concurrency) independent of
which unit a slot belongs to.

Engine split: PE projects kT/qT; DVE+GPSIMD build preact via
tensor_scalar_add (per-partition scalar q column); ACT does tanh in 2S long
N=8192 calls (the bottleneck, ~71us for S=5) and the final exp; PE
accumulates scores via one-hot wv matmuls and computes attn @ values.
"""

import numpy as np
import ml_dtypes
from contextlib import ExitStack

from concourse import bacc, tile
import concourse.bass as bass
import concourse.mybir as mybir
from concourse.bass_utils import run_bass_kernel_spmd

F32 = mybir.dt.float32
F32R = mybir.dt.float32r
BF16 = mybir.dt.bfloat16
AF = mybir.ActivationFunctionType
ts = bass.ts

B, Lq, Lk, D, H = 8, 128, 512, 256, 256
NCORES = 8
JB = 128           # j-block width
NSPLIT = 42        # TS preact calls per (slot, t): l < NSPLIT on DVE, rest GPSIMD

_CACHE = {}


def i_phys(u, l):
    return 32 * (l % 4) + 16 * u + l // 4


def build_program(S):
    nc = bacc.Bacc(
        "TRN2", target_bir_lowering=False, debug=False, enable_asserts=False
    )

    W = S * JB          # scores width
    qsT_d = nc.dram_tensor("qsT", [D, S * 64], F32, kind="ExternalInput")
    keysT_d = nc.dram_tensor("keysT", [D, W], F32R, kind="ExternalInput")
    values_d = nc.dram_tensor("values", [W, H], F32R, kind="ExternalInput")
    Wq_d = nc.dram_tensor("Wq", [D, H], F32, kind="ExternalInput")
    Wk_d = nc.dram_tensor("Wk", [D, H], F32R, kind="ExternalInput")
    mask_d = nc.dram_tensor("mask", [128, W], BF16, kind="ExternalInput")
    identb_d = nc.dram_tensor("identb", [128, 128], BF16, kind="ExternalInput")
    ident_d = nc.dram_tensor("ident", [128, 128], F32, kind="ExternalInput")
    # wv32[(s*2+t)*128 + k, l*32 + r] = wv[t*128+k] iff r == 16*u(s) + l//4
    wv32_d = nc.dram_tensor("wv32", [S * 2 * 128, 64 * 32], BF16, kind="ExternalInput")
    out_d = nc.dram_tensor("out", [Lq, H], F32, kind="ExternalOutput")

    HW2 = W // 2        # col-half of the gathered keys (bank-safe matmul N)

    with tile.TileContext(nc) as tc, ExitStack() as ctx:
        const = ctx.enter_context(tc.tile_pool(name="const", bufs=1))
        inp = ctx.enter_context(tc.tile_pool(name="inp", bufs=1))
        proj = ctx.enter_context(tc.tile_pool(name="proj", bufs=1))
        prep = ctx.enter_context(tc.tile_pool(name="prep", bufs=3))
        featp = ctx.enter_context(tc.tile_pool(name="featp", bufs=3))
        wvp = ctx.enter_context(tc.tile_pool(name="wvp", bufs=4))
        sm = ctx.enter_context(tc.tile_pool(name="sm", bufs=1))
        ps_big = ctx.enter_context(tc.tile_pool(name="ps_big", bufs=1, space="PSUM"))
        ps_sc = ctx.enter_context(tc.tile_pool(name="ps_sc", bufs=1, space="PSUM"))
        ps_sm = ctx.enter_context(tc.tile_pool(name="ps_sm", bufs=1, space="PSUM"))

        # ---- ACT spline table warmup (tanh/exp share a set); no DMA dep ----
        warm_in = sm.tile([1, 2], F32)
        nc.vector.memset(warm_in[:], 0.0)
        warm_sb = sm.tile([1, 2], F32)
        nc.scalar.activation(warm_sb[0:1, 0:1], warm_in[0:1, 0:1], AF.Tanh)
        nc.scalar.activation(warm_sb[0:1, 1:2], warm_in[0:1, 0:1], AF.Exp)

        # ---- input loads; k-projection path first (it gates the pipeline) ----
        keysT_sb = inp.tile([128, 2 * W], F32R)  # [d', dt*W + j]
        for dt in range(2):
            for jh in range(2):
                eng = nc.sync if jh == 0 else nc.scalar
                eng.dma_start(
                    keysT_sb[:, dt * W + jh * HW2 : dt * W + jh * HW2 + HW2],
                    keysT_d[ts(dt, 128), ts(jh, HW2)],
                )
        Wk_sb = inp.tile([128, 2 * H], F32R)  # [d', dt*256 + h]
        for dt in range(2):
            nc.gpsimd.dma_start(Wk_sb[:, ts(dt, H)], Wk_d[ts(dt, 128), :])
        qsT_sb = inp.tile([128, 2 * S * 64], F32)  # [d', dt*(S*64) + c]
        for dt in range(2):
            nc.scalar.dma_start(qsT_sb[:, ts(dt, S * 64)], qsT_d[ts(dt, 128), :])
        Wq_sb = inp.tile([128, 2 * H], F32)
        for dt in range(2):
            nc.sync.dma_start(Wq_sb[:, ts(dt, H)], Wq_d[ts(dt, 128), :])
        mask_sb = const.tile([128, W], BF16)
        nc.gpsimd.dma_start(mask_sb[:], mask_d[:])
        identb_sb = const.tile([128, 128], BF16)
        nc.gpsimd.dma_start(identb_sb[:], identb_d[:])
        ident_sb = const.tile([128, 128], F32)
        nc.scalar.dma_start(ident_sb[:], ident_d[:])
        values_r = inp.tile([128, S * H], F32R)  # [j', s*256 + v]
        for s in range(S):
            nc.gpsimd.dma_start(values_r[:, ts(s, H)], values_d[ts(s, 128), :])

        # ---- projections ----
        # kT_ps laid out [t*1024 + jh*512 .. +HW2] to keep every matmul's
        # output inside one PSUM bank
        kT_ps = ps_big.tile([128, 2048], F32, tag="big")
        for t in range(2):
            for jh in range(2):
                for dt in range(2):
                    nc.tensor.matmul(
                        kT_ps[:, t * 1024 + jh * 512 : t * 1024 + jh * 512 + HW2],
                        Wk_sb[:, dt * H + t * 128 : dt * H + t * 128 + 128],
                        keysT_sb[:, dt * W + jh * HW2 : dt * W + jh * HW2 + HW2],
                        start=(dt == 0),
                        stop=(dt == 1),
                    )
        kT_sb = proj.tile([128, 2 * W], BF16)  # [h', t*W + j]
        for t in range(2):
            for jh in range(2):
                nc.scalar.copy(
                    kT_sb[:, t * W + jh * HW2 : t * W + jh * HW2 + HW2],
                    kT_ps[:, t * 1024 + jh * 512 : t * 1024 + jh * 512 + HW2],
                )

        qT_ps = ps_sm.tile([128, 2 * 512], F32, tag="qt")
        for t in range(2):
            for dt in range(2):
                nc.tensor.matmul(
                    qT_ps[:, t * 512 : t * 512 + S * 64],
                    Wq_sb[:, dt * H + t * 128 : dt * H + t * 128 + 128],
                    qsT_sb[:, ts(dt, S * 64)],
                    start=(dt == 0),
                    stop=(dt == 1),
                )
        qT_sb = proj.tile([128, 2 * S * 64], F32)  # [h', t*(S*64) + s*64 + l]
        for t in range(2):
            nc.scalar.copy(qT_sb[:, ts(t, S * 64)], qT_ps[:, t * 512 : t * 512 + S * 64])

        # ---- scores accumulator; per-row masked init (identity matmul) ----
        sc_ps = ps_sc.tile([128, W], F32)
        nc.tensor.matmul(
            sc_ps[:, 0:512], identb_sb[:], mask_sb[:, 0:512],
            start=True, stop=False, skip_group_check=True,
        )
        if W > 512:
            nc.tensor.matmul(
                sc_ps[:, 512:W], identb_sb[:], mask_sb[:, 512:W],
                start=True, stop=False, skip_group_check=True,
            )

        # ---- main loop over slots ----
        w32s = {}

        def load_w32(s, t):
            w = wvp.tile([128, 64 * 32], BF16, tag="w32")
            nc.sync.dma_start(w[:], wv32_d[ts(s * 2 + t, 128), :])
            w32s[(s, t)] = w

        feats = {}

        def emit_slot(s):
            for t in range(2):
                pre = prep.tile([128, 64 * JB], BF16, tag="pre")
                for l in range(64):
                    eng = nc.vector if l < NSPLIT else nc.gpsimd
                    eng.tensor_scalar_add(
                        pre[:, ts(l, JB)],
                        kT_sb[:, t * W + s * JB : t * W + s * JB + JB],
                        qT_sb[:, t * S * 64 + s * 64 + l : t * S * 64 + s * 64 + l + 1],
                    )
                feat = featp.tile([128, 64 * JB], BF16, tag="feat")
                if (s == 0 and t == 0) or (s == S - 1 and t == 1):
                    for q4 in range(4):
                        nc.scalar.activation(
                            feat[:, ts(q4, 16 * JB)], pre[:, ts(q4, 16 * JB)], AF.Tanh
                        )
                else:
                    nc.scalar.activation(feat[:], pre[:], AF.Tanh)
                feats[(s, t)] = feat

        def emit_acc(s):
            for t in range(2):
                feat = feats.pop((s, t))
                w32 = w32s.pop((s, t))
                for l in range(64):
                    g = l % 4
                    nc.tensor.matmul(
                        sc_ps[32 * g : 32 * g + 32, ts(s, JB)],
                        w32[:, ts(l, 32)],
                        feat[:, ts(l, JB)],
                        start=False,
                        stop=False,
                        tile_position=(0, 32 * g),
                        skip_group_check=True,
                    )

        load_w32(0, 0)
        load_w32(0, 1)
        for s in range(S):
            if s + 1 < S:
                load_w32(s + 1, 0)
                load_w32(s + 1, 1)
            emit_slot(s)
            emit_acc(s)

        # ---- softmax over the gathered j axis (no max-subtraction) ----
        p_sb = sm.tile([128, W], F32)
        sumexp = sm.tile([128, 1], F32)
        nc.scalar.activation(p_sb[:], sc_ps[:], AF.Exp, accum_out=sumexp[:])
        rinv = sm.tile([128, 1], F32)
        nc.vector.reciprocal(rinv[:], sumexp[:])

        # ---- attn @ values ----
        pT_ps = ps_big.tile([128, W], F32, tag="big")
        for s in range(S):
            nc.tensor.transpose(
                pT_ps[:, ts(s, 128)], p_sb[:, ts(s, 128)], ident_sb[:]
            )
        pT_sb = sm.tile([128, W], F32R)  # [j', s*128 + i]
        nc.vector.tensor_copy(pT_sb[:], pT_ps[:])

        out_ps = ps_sm.tile([128, H], F32, tag="qt")
        for s in range(S):
            nc.tensor.matmul(
                out_ps[:],
                pT_sb[:, ts(s, 128)],
                values_r[:, ts(s, H)],
                start=(s == 0),
                stop=(s == S - 1),
            )
        out_sb = sm.tile([128, H], F32)
        nc.vector.tensor_scalar_mul(out_sb[:], out_ps[:], rinv[:])
        nc.sync.dma_start(out_d[:], out_sb[:])

    nc.compile()
    return nc


def _get_program(S):
    if S not in _CACHE:
        _CACHE[S] = build_program(S)
    return _CACHE[S]


def make_schedule(valid_lens):
    """Pack 16 (batch, row-half) units, sizes ceil(vl/128), into 8 cores of
    equal slot count S (largest-with-smallest pairing; pad short cores)."""
    vl = np.asarray(valid_lens).astype(np.int64).reshape(B)
    jb = [max(1, int(-(-v // JB))) for v in vl]
    units = [(b, h) for b in range(B) for h in range(2)]
    order = sorted(range(16), key=lambda idx: -jb[units[idx][0]])
    pairs = [(units[order[k]], units[order[15 - k]]) for k in range(8)]
    S = max(jb[a[0]] + jb[bu[0]] for a, bu in pairs)
    # schedule[core] = (unitA, unitB, slots) with slots = list of
    # (u, b, jblock) or None for pad
    schedule = []
    for uA, uB in pairs:
        slots = [(0, uA[0], k) for k in range(jb[uA[0]])] + [
            (1, uB[0], k) for k in range(jb[uB[0]])
        ]
        slots += [None] * (S - len(slots))
        schedule.append((uA, uB, slots))
    return S, schedule


def make_in_maps(queries, keys, values, valid_lens, Wq, Wk, wv):
    queries = np.ascontiguousarray(queries, dtype=np.float32)
    keys = np.ascontiguousarray(keys, dtype=np.float32)
    values = np.ascontiguousarray(values, dtype=np.float32)
    Wq = np.ascontiguousarray(Wq, dtype=np.float32)
    Wk = np.ascontiguousarray(Wk, dtype=np.float32)
    wv = np.ascontiguousarray(wv, dtype=np.float32).reshape(H)
    vl = np.asarray(valid_lens).astype(np.int64).reshape(B)
    S, schedule = make_schedule(vl)
    W = S * JB
    bf = ml_dtypes.bfloat16
    ident = np.eye(128, dtype=np.float32)
    identb = np.eye(128, dtype=bf)
    wvb = wv.astype(bf)
    jj = np.arange(JB)
    in_maps = []
    for core in range(NCORES):
        uA, uB, slots = schedule[core]
        keysT_c = np.zeros((D, W), dtype=np.float32)
        values_c = np.zeros((W, H), dtype=np.float32)
        mask_c = np.full((128, W), -1e6, dtype=np.float32)
        wv32_c = np.zeros((S, 2, 128, 64, 32), dtype=bf)
        qslot = np.zeros((S * 64, D), dtype=np.float32)
        for s, slot in enumerate(slots):
            if slot is None:
                continue
            u, b, k = slot
            half = uA[1] if u == 0 else uB[1]
            keysT_c[:, s * JB : (s + 1) * JB] = keys[b, k * JB : (k + 1) * JB, :].T
            values_c[s * JB : (s + 1) * JB, :] = values[b, k * JB : (k + 1) * JB, :]
            qslot[s * 64 : (s + 1) * 64, :] = queries[b, half * 64 : half * 64 + 64, :]
            valid = np.minimum(np.maximum(vl[b] - k * JB, 0), JB)
            col_ok = jj < valid  # (128,)
            rows = np.array([i_phys(u, l) for l in range(64)])
            mask_c[rows[:, None], s * JB + jj[None, :]] = np.where(
                col_ok[None, :], 0.0, -1e6
            )
            for t in range(2):
                ll = np.arange(64)
                wv32_c[s, t, :, ll, 16 * u + ll // 4] = wvb[t * 128 : (t + 1) * 128]
        in_maps.append(
            {
                "qsT": np.ascontiguousarray(qslot.T),
                "keysT": keysT_c,
                "values": values_c,
                "Wq": Wq,
                "Wk": Wk,
                "mask": mask_c.astype(bf),
                "identb": identb,
                "ident": ident,
                "wv32": wv32_c.reshape(S * 2 * 128, 64 * 32),
            }
        )
    return S, schedule, in_maps


def assemble(schedule, core_outs):
    out = np.zeros((B, Lq, H), dtype=np.float32)
    for core in range(NCORES):
        uA, uB, _ = schedule[core]
        oc = core_outs[core]
        for u, (b, half) in ((0, uA), (1, uB)):
            for l in range(64):
                out[b, half * 64 + l, :] = oc[i_phys(u, l), :]
    return out


def kernel(**inputs):
    S, schedule, in_maps = make_in_maps(
        inputs["queries"],
        inputs["keys"],
        inputs["values"],
        inputs["valid_lens"],
        inputs["Wq"],
        inputs["Wk"],
        inputs["wv"],
    )
    nc = _get_program(S)
    res = run_bass_kernel_spmd(nc, in_maps, core_ids=list(range(NCORES)))
    return assemble(schedule, [res.results[c]["out"] for c in range(NCORES)])


# revision 21
# speedup vs baseline: 4.4983x; 4.4983x over previous
"""AdditiveAttention on Trainium2 (Bass/Tile), 8 cores, valid_lens-aware resharding.

Reference per batch b:
  q = queries @ Wq; k = keys @ Wk
  scores[i,j] = wv . tanh(q[i] + k[j]); masked softmax over j; out = attn @ values

Masked columns (j >= valid_len) contribute exactly zero, so only
ceil(valid_len/256) 256-wide j-slots per batch need computing. Work units of
(batch, 64 query rows) are packed two per core (unit A -> u=0, unit B -> u=1)
with S_A slots for A and S_B for B (largest-with-smallest pairing; short
units get zero-key, -1e6-masked pad slots). One SPMD program: all per-core
variation lives in input DATA (slot-gathered keys/values/mask, stacked
queries); instructions are identical on every core.

Row permutation i_phys(u, l) = 32*(l%4) + 16*u + l//4 puts consecutive l on
disjoint PE column groups (4-way tile_position concurrency); the one-hot
column of the wv stationary (shared constant data) routes each matmul's
result to that row.

Engine split: PE projects kT/qT; DVE builds preact via tensor_scalar_add
(per-partition scalar q column, FD = S_A*256 per call via slot-interleaved
layout); ACT does tanh in 2*(2+S_B... ) long N=16K calls (the bottleneck)
plus the final exp (no max-subtraction: |scores| <= sum|wv| ~ 13); PE
accumulates scores via one-hot wv matmuls and computes attn @ values.
"""

import numpy as np
import ml_dtypes
from contextlib import ExitStack

from concourse import bacc, tile
import concourse.bass as bass
import concourse.mybir as mybir
from concourse.bass_utils import run_bass_kernel_spmd

F32 = mybir.dt.float32
F32R = mybir.dt.float32r
BF16 = mybir.dt.bfloat16
AF = mybir.ActivationFunctionType
ts = bass.ts

B, Lq, Lk, D, H = 8, 128, 512, 256, 256
NCORES = 8
JB = 256           # j-slot width

_CACHE = {}


def i_phys(u, l):
    return 32 * (l % 4) + 16 * u + l // 4


def build_program(SA, SB):
    nc = bacc.Bacc(
        "TRN2", target_bir_lowering=False, debug=False, enable_asserts=False
    )

    S = SA + SB
    W = S * JB          # gathered scores width
    WA, WB = SA * JB, SB * JB
    qsT_d = nc.dram_tensor("qsT", [D, 128], F32, kind="ExternalInput")
    keysT_d = nc.dram_tensor("keysT", [D, W], F32R, kind="ExternalInput")
    values_d = nc.dram_tensor("values", [W, H], F32R, kind="ExternalInput")
    Wq_d = nc.dram_tensor("Wq", [D, H], F32, kind="ExternalInput")
    Wk_d = nc.dram_tensor("Wk", [D, H], F32R, kind="ExternalInput")
    mask_d = nc.dram_tensor("mask", [128, W], BF16, kind="ExternalInput")
    identb_d = nc.dram_tensor("identb", [128, 128], BF16, kind="ExternalInput")
    ident_d = nc.dram_tensor("ident", [128, 128], F32, kind="ExternalInput")
    # wv32[(u*2+t)*128 + k, l*32 + r] = wv[t*128+k] iff r == 16*u + l//4
    wv32_d = nc.dram_tensor("wv32", [2 * 2 * 128, 64 * 32], BF16, kind="ExternalInput")
    out_d = nc.dram_tensor("out", [Lq, H], F32, kind="ExternalOutput")

    NJ6 = W // 128      # 128-row j-blocks of the gathered axis (for attn@values)

    with tile.TileContext(nc) as tc, ExitStack() as ctx:
        const = ctx.enter_context(tc.tile_pool(name="const", bufs=1))
        inp = ctx.enter_context(tc.tile_pool(name="inp", bufs=1))
        proj = ctx.enter_context(tc.tile_pool(name="proj", bufs=1))
        prep = ctx.enter_context(tc.tile_pool(name="prep", bufs=2))
        featp = ctx.enter_context(tc.tile_pool(name="featp", bufs=2))
        sm = ctx.enter_context(tc.tile_pool(name="sm", bufs=1))
        ps_big = ctx.enter_context(tc.tile_pool(name="ps_big", bufs=1, space="PSUM"))
        ps_sc = ctx.enter_context(tc.tile_pool(name="ps_sc", bufs=1, space="PSUM"))
        ps_sm = ctx.enter_context(tc.tile_pool(name="ps_sm", bufs=1, space="PSUM"))

        # ---- ACT spline table warmup (tanh/exp share a set); no DMA dep ----
        warm_in = sm.tile([1, 2], F32)
        nc.vector.memset(warm_in[:], 0.0)
        warm_sb = sm.tile([1, 2], F32)
        nc.scalar.activation(warm_sb[0:1, 0:1], warm_in[0:1, 0:1], AF.Tanh)
        nc.scalar.activation(warm_sb[0:1, 1:2], warm_in[0:1, 0:1], AF.Exp)

        # ---- input loads; k-projection path first (it gates the pipeline) ----
        HW2 = W // 2
        keysT_sb = inp.tile([128, 2 * W], F32R)  # [d', dt*W + j]
        for dt in range(2):
            for jh in range(2):
                eng = nc.sync if jh == 0 else nc.scalar
                eng.dma_start(
                    keysT_sb[:, dt * W + jh * HW2 : dt * W + jh * HW2 + HW2],
                    keysT_d[ts(dt, 128), ts(jh, HW2)],
                )
        Wk_sb = inp.tile([128, 2 * H], F32R)  # [d', dt*256 + h]
        for dt in range(2):
            nc.gpsimd.dma_start(Wk_sb[:, ts(dt, H)], Wk_d[ts(dt, 128), :])
        qsT_sb = inp.tile([128, D], F32)  # [d', dt*128 + (u*64+l)]
        for dt in range(2):
            nc.scalar.dma_start(qsT_sb[:, ts(dt, 128)], qsT_d[ts(dt, 128), :])
        Wq_sb = inp.tile([128, 2 * H], F32)
        for dt in range(2):
            nc.sync.dma_start(Wq_sb[:, ts(dt, H)], Wq_d[ts(dt, 128), :])
        mask_sb = const.tile([128, W], BF16)
        nc.gpsimd.dma_start(mask_sb[:], mask_d[:])
        identb_sb = const.tile([128, 128], BF16)
        nc.gpsimd.dma_start(identb_sb[:], identb_d[:])
        ident_sb = const.tile([128, 128], F32)
        nc.scalar.dma_start(ident_sb[:], ident_d[:])
        w32_sb = const.tile([128, 4 * 64 * 32], BF16)  # [(u*2+t) blocks]
        for ut in range(4):
            nc.sync.dma_start(w32_sb[:, ts(ut, 64 * 32)], wv32_d[ts(ut, 128), :])
        values_r = inp.tile([128, NJ6 * H], F32R)  # [j', jb*256 + v]
        for jb in range(NJ6):
            nc.gpsimd.dma_start(values_r[:, ts(jb, H)], values_d[ts(jb, 128), :])

        # ---- projections ----
        # kT_ps laid out [t*1024 + jh*512 .. +W/2] so no matmul output
        # crosses a PSUM bank boundary
        kT_ps = ps_big.tile([128, 2048], F32, tag="big")
        for t in range(2):
            for jh in range(2):
                for dt in range(2):
                    nc.tensor.matmul(
                        kT_ps[:, t * 1024 + jh * 512 : t * 1024 + jh * 512 + HW2],
                        Wk_sb[:, dt * H + t * 128 : dt * H + t * 128 + 128],
                        keysT_sb[:, dt * W + jh * HW2 : dt * W + jh * HW2 + HW2],
                        start=(dt == 0),
                        stop=(dt == 1),
                    )
        kT_sb = proj.tile([128, 2 * W], BF16)  # [h', t*W + j]
        for t in range(2):
            for jh in range(2):
                nc.scalar.copy(
                    kT_sb[:, t * W + jh * HW2 : t * W + jh * HW2 + HW2],
                    kT_ps[:, t * 1024 + jh * 512 : t * 1024 + jh * 512 + HW2],
                )

        qT_ps = ps_sm.tile([128, 2 * 128], F32, tag="qt")
        for t in range(2):
            for dt in range(2):
                nc.tensor.matmul(
                    qT_ps[:, ts(t, 128)],
                    Wq_sb[:, dt * H + t * 128 : dt * H + t * 128 + 128],
                    qsT_sb[:, ts(dt, 128)],
                    start=(dt == 0),
                    stop=(dt == 1),
                )
        qT_sb = proj.tile([128, 2 * 128], F32)  # [h', t*128 + u*64 + l]
        nc.scalar.copy(qT_sb[:], qT_ps[:])

        # ---- scores accumulator; per-row masked init (identity matmul) ----
        sc_ps = ps_sc.tile([128, W], F32)
        for jh in range((W + 511) // 512):
            hi = min(W, jh * 512 + 512)
            nc.tensor.matmul(
                sc_ps[:, jh * 512 : hi], identb_sb[:], mask_sb[:, jh * 512 : hi],
                start=True, stop=False, skip_group_check=True,
            )

        # ---- main loop: 6 tile-groups ----
        # unit A (slots 0..SA-1, slot-interleaved per row: FD=SA*256 preacts):
        #   tiles (t, lh): pre[h', (l-32*lh)*WA + s*256 + j'], l in [32lh, 32lh+32)
        # unit B (slots SA..S-1): tiles (t): pre[h', l*WB + s'*256 + j']
        def emit_groupA(t, lh, split):
            pre = prep.tile([128, 32 * WA], BF16, tag="pre")
            for l in range(32 * lh, 32 * lh + 32):
                nc.vector.tensor_scalar_add(
                    pre[:, (l - 32 * lh) * WA : (l - 32 * lh + 1) * WA],
                    kT_sb[:, t * W : t * W + WA],
                    qT_sb[:, t * 128 + l : t * 128 + l + 1],
                )
            feat = featp.tile([128, 32 * WA], BF16, tag="feat")
            if split:
                for q4 in range(4):
                    nc.scalar.activation(
                        feat[:, ts(q4, 8 * WA)], pre[:, ts(q4, 8 * WA)], AF.Tanh
                    )
            else:
                nc.scalar.activation(feat[:], pre[:], AF.Tanh)
            # accumulate: s-outer, l-inner so consecutive matmuls rotate the
            # 4 PE column groups
            for s in range(SA):
                for l in range(32 * lh, 32 * lh + 32):
                    g = l % 4
                    nc.tensor.matmul(
                        sc_ps[32 * g : 32 * g + 32, s * JB : s * JB + JB],
                        w32_sb[:, t * 2048 + l * 32 : t * 2048 + l * 32 + 32],
                        feat[:, (l - 32 * lh) * WA + s * JB : (l - 32 * lh) * WA + s * JB + JB],
                        start=False,
                        stop=False,
                        tile_position=(0, 32 * g),
                        skip_group_check=True,
                    )

        def emit_groupB(t, split):
            pre = prep.tile([128, 64 * WB], BF16, tag="pre")
            for l in range(64):
                nc.vector.tensor_scalar_add(
                    pre[:, l * WB : (l + 1) * WB],
                    kT_sb[:, t * W + WA : t * W + WA + WB],
                    qT_sb[:, t * 128 + 64 + l : t * 128 + 64 + l + 1],
                )
            feat = featp.tile([128, 64 * WB], BF16, tag="feat")
            if split:
                for q4 in range(4):
                    nc.scalar.activation(
                        feat[:, ts(q4, 16 * WB)], pre[:, ts(q4, 16 * WB)], AF.Tanh
                    )
            else:
                nc.scalar.activation(feat[:], pre[:], AF.Tanh)
            for s in range(SB):
                for l in range(64):
                    g = l % 4
                    nc.tensor.matmul(
                        sc_ps[32 * g : 32 * g + 32, WA + s * JB : WA + s * JB + JB],
                        w32_sb[:, (2 + t) * 2048 + l * 32 : (2 + t) * 2048 + l * 32 + 32],
                        feat[:, l * WB + s * JB : l * WB + s * JB + JB],
                        start=False,
                        stop=False,
                        tile_position=(0, 32 * g),
                        skip_group_check=True,
                    )

        emit_groupA(0, 0, split=True)
        emit_groupA(0, 1, split=False)
        emit_groupA(1, 0, split=False)
        emit_groupA(1, 1, split=False)
        emit_groupB(0, split=False)
        emit_groupB(1, split=True)

        # ---- softmax over the gathered j axis (no max-subtraction) ----
        p_sb = sm.tile([128, W], F32)
        sumexp = sm.tile([128, 1], F32)
        nc.scalar.activation(p_sb[:], sc_ps[:], AF.Exp, accum_out=sumexp[:])
        rinv = sm.tile([128, 1], F32)
        nc.vector.reciprocal(rinv[:], sumexp[:])

        # ---- attn @ values ----
        pT_ps = ps_big.tile([128, NJ6 * 128], F32, tag="big")
        for jb in range(NJ6):
            nc.tensor.transpose(
                pT_ps[:, ts(jb, 128)], p_sb[:, ts(jb, 128)], ident_sb[:]
            )
        pT_sb = sm.tile([128, NJ6 * 128], F32R)  # [j', jb*128 + i]
        nc.vector.tensor_copy(pT_sb[:], pT_ps[:])

        out_ps = ps_sm.tile([128, H], F32, tag="qt")
        for jb in range(NJ6):
            nc.tensor.matmul(
                out_ps[:],
                pT_sb[:, ts(jb, 128)],
                values_r[:, ts(jb, H)],
                start=(jb == 0),
                stop=(jb == NJ6 - 1),
            )
        out_sb = sm.tile([128, H], F32)
        nc.vector.tensor_scalar_mul(out_sb[:], out_ps[:], rinv[:])
        nc.sync.dma_start(out_d[:], out_sb[:])

    nc.compile()
    return nc


def _get_program(key):
    if key not in _CACHE:
        _CACHE[key] = build_program(*key)
    return _CACHE[key]


def make_schedule(valid_lens):
    """Pack 16 (batch, row-half) units, sizes ceil(vl/256), two per core
    (largest-with-smallest pairing). Returns (SA, SB, schedule) where
    schedule[core] = ((bA, halfA, jbA), (bB, halfB, jbB))."""
    vl = np.asarray(valid_lens).astype(np.int64).reshape(B)
    jb = [max(1, int(-(-v // JB))) for v in vl]
    units = [(b, h, jb[b]) for b in range(B) for h in range(2)]
    order = sorted(range(16), key=lambda idx: -units[idx][2])
    pairs = [(units[order[k]], units[order[15 - k]]) for k in range(8)]
    SA = max(p[0][2] for p in pairs)
    SB = max(p[1][2] for p in pairs)
    return SA, SB, pairs


def make_in_maps(queries, keys, values, valid_lens, Wq, Wk, wv):
    queries = np.ascontiguousarray(queries, dtype=np.float32)
    keys = np.ascontiguousarray(keys, dtype=np.float32)
    values = np.ascontiguousarray(values, dtype=np.float32)
    Wq = np.ascontiguousarray(Wq, dtype=np.float32)
    Wk = np.ascontiguousarray(Wk, dtype=np.float32)
    wv = np.ascontiguousarray(wv, dtype=np.float32).reshape(H)
    vl = np.asarray(valid_lens).astype(np.int64).reshape(B)
    SA, SB, schedule = make_schedule(vl)
    S = SA + SB
    W = S * JB
    bf = ml_dtypes.bfloat16
    ident = np.eye(128, dtype=np.float32)
    identb = np.eye(128, dtype=bf)
    wvb = wv.astype(bf)
    # shared one-hot wv stationaries: block (u, t)
    wv32 = np.zeros((2, 2, 128, 64, 32), dtype=bf)
    ll = np.arange(64)
    for u in range(2):
        for t in range(2):
            wv32[u, t, :, ll, 16 * u + ll // 4] = wvb[t * 128 : (t + 1) * 128]
    wv32 = wv32.reshape(4 * 128, 64 * 32)
    jj = np.arange(JB)
    in_maps = []
    for core in range(NCORES):
        uA, uB = schedule[core]
        keysT_c = np.zeros((D, W), dtype=np.float32)
        values_c = np.zeros((W, H), dtype=np.float32)
        mask_c = np.full((128, W), -1e6, dtype=np.float32)
        qstack = np.zeros((128, D), dtype=np.float32)
        for u, (b, half, jbu), s0, su in ((0, uA, 0, SA), (1, uB, SA, SB)):
            qstack[u * 64 : u * 64 + 64, :] = queries[b, half * 64 : half * 64 + 64, :]
            rows = np.array([i_phys(u, l) for l in range(64)])
            for k in range(min(jbu, su)):
                s = s0 + k
                keysT_c[:, s * JB : (s + 1) * JB] = keys[b, k * JB : (k + 1) * JB, :].T
                values_c[s * JB : (s + 1) * JB, :] = values[b, k * JB : (k + 1) * JB, :]
                valid = np.minimum(np.maximum(vl[b] - k * JB, 0), JB)
                mask_c[rows[:, None], s * JB + jj[None, :]] = np.where(
                    (jj < valid)[None, :], 0.0, -1e6
                )
        in_maps.append(
            {
                "qsT": np.ascontiguousarray(qstack.T),
                "keysT": keysT_c,
                "values": values_c,
                "Wq": Wq,
                "Wk": Wk,
                "mask": mask_c.astype(bf),
                "identb": identb,
                "ident": ident,
                "wv32": wv32,
            }
        )
    return (SA, SB), schedule, in_maps


def assemble(schedule, core_outs):
    out = np.zeros((B, Lq, H), dtype=np.float32)
    for core in range(NCORES):
        uA, uB = schedule[core]
        oc = core_outs[core]
        for u, (b, half, _) in ((0, uA), (1, uB)):
            for l in range(64):
                out[b, half * 64 + l, :] = oc[i_phys(u, l), :]
    return out


def kernel(**inputs):
    key, schedule, in_maps = make_in_maps(
        inputs["queries"],
        inputs["keys"],
        inputs["values"],
        inputs["valid_lens"],
        inputs["Wq"],
        inputs["Wk"],
        inputs["wv"],
    )
    nc = _get_program(key)
    res = run_bass_kernel_spmd(nc, in_maps, core_ids=list(range(NCORES)))
    return assemble(schedule, [res.results[c]["out"] for c in range(NCORES)])
